# revision 6
# baseline (speedup 1.0000x reference)
"""AdaAttention (gumbel-gated sparse attention block) on 8 TRN2 NeuronCores.

Strategy: pure data-parallel over batch (64 batches -> 8 per core).  Each
core runs the full attention block for its 8 batches; no collectives.

Per-core layout (all f32):
  xt   [768, 1576]  x^T for this core's 8 batches (host transposes)
  wall [768, 2305]  concat([Wq, Wk, Wv, Wm], 0).T  (weights as lhsT tiles)
  wpt  [768, 768]   Wp.T
  bpt  [6, 128]     bp reshaped per 128-row output tile
  thr  [1, 1576]    gating threshold per token: 5*ln(1.5) - g1 + g2
                    (CLS slots = -1e30 so the CLS gate always passes)
  outt [768, 1576]  out^T (host transposes back)

Device pipeline:
  1. logits = Wm @ x^T (PE), ts = (logits > thr) via DVE is_gt; ts is
     round-tripped through DRAM to get per-partition column layout.
  2. QKV GEMM, transposed outputs: Q^T (pre-scaled by 1/8), K^T (spilled
     to DRAM, re-streamed per head-pair), V^T (doubles as the attention
     output buffer attnT: gated tokens pass v[n] through unchanged).
  3. V in normal layout ([token, head*65] with a ones column appended)
     for the PV matmul / softmax denominator.
  4. Per (batch, head): S^T = K_h @ Q_h^T (PE, odd/even heads land on
     row-groups 0/64 for concurrency); fused mask+exp in ONE ScalarE
     pass: exp(S*ts_m + 30*ts_m - 30) with per-partition scale/bias;
     PV^T+Z via [V|1] stationary; Z broadcast via a 1-row ones matmul;
     divide (DVE); copy_predicated overwrites kept-token columns of
     attnT (gated columns keep V^T).
  5. proj GEMM + bias, DMA out.
"""
import sys
import types

import numpy as np

# ---------------------------------------------------------------- patches
# This container's walrus rejects >1 sync-wait per instruction; Tile's
# kernel-tail drain aggregates one wait per outstanding proc.  Spread the
# waits across one sync-NOP each.  Also: the image's antenv lacks
# axon_hooks (NTFF profiling) and upload_artifacts wants a bucket.


def _install_patches():
    import bass_rust
    import concourse.tile as tile

    def _patched_drain_and_barrier(self, tick_clock, wait_clock):
        gc = tick_clock.global_clock
        ticks = eval(repr(gc).replace("VectorClock(", "").rstrip(")"))
        for i, t in enumerate(ticks):
            if t > 0:
                cur = list(ticks)
                cur[i] = 0
                nop = self.nc.sync.nop()
                wait_clock.add_sem_waits(
                    nop.ins,
                    tile.ScopedClock({None: gc}),
                    tile.ScopedClock({None: bass_rust.VectorClock(cur)}),
                )
        drain_inst = self.nc.sync.drain()
        wait_clock.add_sem_waits(
            drain_inst.ins, tile.ScopedClock({None: gc}), tile.ScopedClock({None: gc})
        )
        self.nc.all_engine_barrier()
        assert self.sems is not None
        popped = self.nc._tile_sem_poison_stack.pop()
        assert popped is self._sem_poison
        self.nc.clear_and_free_semaphores(list(self.sems.allocated().values()))
        self.nc.all_engine_barrier()

    tile.TileContext._drain_and_barrier = _patched_drain_and_barrier

    if "antenv.axon_hooks" not in sys.modules:
        mod = types.ModuleType("antenv.axon_hooks")
        try:
            from trn_agent_boot.trn_boot import _ntff_profile_via_ctypes

            hook = _ntff_profile_via_ctypes("/opt/axon/libaxon_pjrt.so")
        except Exception:
            hook = None
        mod.get_axon_ntff_profile_hook = lambda: hook
        mod.set_axon_ntff_profile_hook = lambda h: None
        sys.modules["antenv.axon_hooks"] = mod

    import concourse.bass_utils as bass_utils

    bass_utils.upload_artifacts = lambda tmpdir: f"file://{tmpdir}"


_install_patches()

import concourse.bass as bass  # noqa: E402
import concourse.mybir as mybir  # noqa: E402
import concourse.tile as tile  # noqa: E402
from concourse.bass_utils import run_bass_kernel_spmd  # noqa: E402


def _split_multi_waits(nc):
    """walrus here allows one sync-wait per engine instruction: hoist extra
    waits onto same-engine NoOps inserted immediately before."""
    for fn in nc.m.functions:
        for bb in fn.blocks:
            out = []
            changed = False
            for inst in bb.instructions:
                si = inst.sync_info
                waits = list(si.on_wait) if si is not None else []
                if len(waits) > 1:
                    changed = True
                    for k, w in enumerate(waits[:-1]):
                        nop = mybir.InstNoOp(
                            name=f"{inst.name}-w{k}",
                            engine=inst.engine,
                            ins=[],
                            outs=[],
                            sync_info=mybir.SyncInfo(on_wait=[w], on_update=[]),
                        )
                        out.append(nop)
                    si.on_wait = [waits[-1]]
                out.append(inst)
            if changed:
                try:
                    bb.instructions = out
                except Exception:
                    bb.instructions.clear()
                    bb.instructions.extend(out)

F32 = mybir.dt.float32
AF = mybir.ActivationFunctionType
OP = mybir.AluOpType

B, N, C = 64, 197, 768
H, D = 12, 64
NCORES = 8
BL = B // NCORES  # 8 batches per core
T = BL * N  # 1576 tokens per core
TCH = 394  # token chunk (= 2 batches); 4 chunks per core
NEG = 30.0  # masked scores get exp(-30)


def _chunks(b):
    """(start, count) partition chunks of batch b's 197 tokens."""
    a = b * N
    return [(a, 128), (a + 128, 69)]


def build_nc():
    nc = bass.Bass()
    xt_d = nc.declare_dram_parameter("xt", [C, T], F32, isOutput=False)
    wall_d = nc.declare_dram_parameter("wall", [C, 3 * C + 1], F32, isOutput=False)
    wpt_d = nc.declare_dram_parameter("wpt", [C, C], F32, isOutput=False)
    bpt_d = nc.declare_dram_parameter("bpt", [6, 128], F32, isOutput=False)
    thr_d = nc.declare_dram_parameter("thr", [1, T], F32, isOutput=False)
    out_d = nc.declare_dram_parameter("outt", [C, T], F32, isOutput=True)

    tsd = nc.dram_tensor("tsd", [1, T], F32, kind="Internal")
    ktd = nc.dram_tensor("ktd", [C, T], F32, kind="Internal")

    with tile.TileContext(nc) as tc:
        from contextlib import ExitStack

        with ExitStack() as ctx:
            E = ctx.enter_context
            cpool = E(tc.tile_pool(name="const", bufs=1))
            xpool = E(tc.tile_pool(name="xt", bufs=6))
            qpool = E(tc.tile_pool(name="qt", bufs=6))
            apool = E(tc.tile_pool(name="at", bufs=6))
            kpool = E(tc.tile_pool(name="kt", bufs=2))
            vpool = E(tc.tile_pool(name="vaug", bufs=16))
            w1pool = E(tc.tile_pool(name="w128", bufs=8))
            w3pool = E(tc.tile_pool(name="w384", bufs=7))
            ppool = E(tc.tile_pool(name="pp", bufs=6))
            zpool = E(tc.tile_pool(name="pvz", bufs=2))
            tbpool = E(tc.tile_pool(name="tsb", bufs=8))
            opool = E(tc.tile_pool(name="out", bufs=2))
            scpool = E(tc.tile_pool(name="smallcol", bufs=40))
            ps_mm = E(tc.tile_pool(name="ps_mm", bufs=2, space="PSUM"))
            ps_s = E(tc.tile_pool(name="ps_s", bufs=2, space="PSUM"))
            ps_pv = E(tc.tile_pool(name="ps_pv", bufs=2, space="PSUM"))
            ps_bc = E(tc.tile_pool(name="ps_bc", bufs=2, space="PSUM"))

            # ---- constants / inputs
            ones = cpool.tile([128, 128], F32)
            nc.vector.memset(ones[:], 1.0)

            xt = []
            for c in range(6):
                t_ = xpool.tile([128, T], F32)
                nc.sync.dma_start(t_[:], xt_d[c * 128 : (c + 1) * 128, :])
                xt.append(t_)

            thr_sb = kpool.tile([1, T], F32, tag="kt")
            nc.sync.dma_start(thr_sb[:], thr_d[:])

            bp_sb = []
            for ct in range(6):
                t_ = scpool.tile([128, 1], F32, tag="bp")
                nc.sync.dma_start(
                    t_[:], bpt_d[ct : ct + 1, :].rearrange("a b -> b a")
                )
                bp_sb.append(t_)

            # ---- gating: logits = Wm @ x^T ; ts = logits > thr
            wlog = []
            for c in range(6):
                t_ = w1pool.tile([128, 1], F32, tag="wlog")
                nc.sync.dma_start(
                    t_[:], wall_d[c * 128 : (c + 1) * 128, 3 * C : 3 * C + 1]
                )
                wlog.append(t_)

            ts_row = kpool.tile([1, T], F32, tag="kt")
            for tch in range(4):
                sl = slice(tch * TCH, (tch + 1) * TCH)
                pl = ps_s.tile([1, TCH], F32, tag="s")
                for c in range(6):
                    nc.tensor.matmul(
                        pl[:], wlog[c][:], xt[c][:, sl], start=(c == 0), stop=(c == 5)
                    )
                nc.vector.tensor_tensor(
                    ts_row[0:1, sl], pl[0:1, :], thr_sb[0:1, sl], op=OP.is_gt
                )
            nc.sync.dma_start(tsd[:], ts_row[:])

            ts_col, bias_col = [], []
            for b in range(BL):
                for (a, cnt) in _chunks(b):
                    tcl = scpool.tile([128, 1], F32, tag="tscol")
                    nc.sync.dma_start(
                        tcl[0:cnt, :], tsd[0:1, a : a + cnt].rearrange("a b -> b a")
                    )
                    bcl = scpool.tile([128, 1], F32, tag="bcol")
                    nc.vector.tensor_scalar(
                        bcl[0:cnt, :], tcl[0:cnt, :], NEG, -NEG, op0=OP.mult, op1=OP.add
                    )
                    ts_col.append(tcl)
                    bias_col.append(bcl)

            # ts broadcast [128,197] per batch (predicate for the blend)
            tsb = []
            for b in range(BL):
                pt = ps_bc.tile([128, N], F32, tag="bc")
                nc.tensor.matmul(
                    pt[:], ones[0:1, 0:128], ts_row[0:1, b * N : (b + 1) * N]
                )
                st = tbpool.tile([128, N], mybir.dt.int32)
                nc.scalar.copy(st[:], pt[:])
                tsb.append(st)

            # ---- QKV GEMM (transposed outputs).  f-tiles: 0-5 Q, 6-11 K,
            # 12-17 V^T (V^T tiles double as attnT).
            qt, at = [], []
            for f in range(18):
                wts = []
                for c in range(6):
                    wt = w1pool.tile([128, 128], F32, tag="w")
                    nc.sync.dma_start(
                        wt[:],
                        wall_d[c * 128 : (c + 1) * 128, f * 128 : (f + 1) * 128],
                    )
                    wts.append(wt)
                if f < 6:
                    dst = qpool.tile([128, T], F32)
                    qt.append(dst)
                elif f < 12:
                    dst = kpool.tile([128, T], F32, tag="kt")
                else:
                    dst = apool.tile([128, T], F32)
                    at.append(dst)
                for tch in range(4):
                    sl = slice(tch * TCH, (tch + 1) * TCH)
                    pm = ps_mm.tile([128, TCH], F32, tag="mm")
                    for c in range(6):
                        nc.tensor.matmul(
                            pm[:], wts[c][:], xt[c][:, sl],
                            start=(c == 0), stop=(c == 5),
                        )
                    if f < 6:
                        nc.scalar.mul(dst[:, sl], pm[:], 0.125)
                    else:
                        nc.vector.tensor_copy(dst[:, sl], pm[:])
                if 6 <= f < 12:
                    g = f - 6
                    nc.sync.dma_start(ktd[g * 128 : (g + 1) * 128, :], dst[:])

            # ---- V in normal layout, augmented with a ones column:
            # vaug[b,j][m, h, 0:64] = V tokens, [..., 64] = 1.0
            vaug = []
            for b in range(BL):
                for j, (a, cnt) in enumerate(_chunks(b)):
                    vt = vpool.tile([128, H, D + 1], F32)
                    nc.vector.memset(vt[:, :, D : D + 1], 1.0)
                    vaug.append(vt)
            for n2 in range(2):
                wv = []
                for c in range(6):
                    wt = w3pool.tile([128, 384], F32, tag="wv")
                    nc.sync.dma_start(
                        wt[:],
                        wall_d[
                            c * 128 : (c + 1) * 128,
                            2 * C + n2 * 384 : 2 * C + (n2 + 1) * 384,
                        ],
                    )
                    wv.append(wt)
                for b in range(BL):
                    for j, (a, cnt) in enumerate(_chunks(b)):
                        pm = ps_mm.tile([128, TCH], F32, tag="mm")
                        for c in range(6):
                            nc.tensor.matmul(
                                pm[0:cnt, 0:384],
                                xt[c][:, a : a + cnt],
                                wv[c][:],
                                start=(c == 0),
                                stop=(c == 5),
                            )
                        nc.scalar.copy(
                            vaug[b * 2 + j][0:cnt, n2 * 6 : (n2 + 1) * 6, 0:D],
                            pm[0:cnt, 0:384].rearrange("p (h d) -> p h d", d=D),
                        )

            # ---- attention, head-pair major (K^T streamed from DRAM)
            for hp in range(6):
                ktt = kpool.tile([128, T], F32, tag="kt")
                nc.sync.dma_start(ktt[:], ktd[hp * 128 : (hp + 1) * 128, :])
                for b in range(BL):
                    bsl = slice(b * N, (b + 1) * N)
                    for hh in range(2):
                        h = hp * 2 + hh
                        hb = hh * 64
                        pps = []
                        for j, (a, cnt) in enumerate(_chunks(b)):
                            ps = ps_s.tile([128, N], F32, tag="s")
                            nc.tensor.matmul(
                                ps[0:cnt, :],
                                ktt[hb : hb + 64, a : a + cnt],
                                qt[hp][hb : hb + 64, bsl],
                            )
                            pp = ppool.tile([128, N], F32, tag="pp")
                            nc.scalar.activation(
                                pp[0:cnt, :],
                                ps[0:cnt, :],
                                AF.Exp,
                                bias=bias_col[b * 2 + j][0:cnt, :],
                                scale=ts_col[b * 2 + j][0:cnt, :],
                            )
                            pps.append(pp)
                        pv = ps_pv.tile([65, N], F32, tag="pv")
                        for j, (a, cnt) in enumerate(_chunks(b)):
                            nc.tensor.matmul(
                                pv[:],
                                vaug[b * 2 + j][0:cnt, h, :],
                                pps[j][0:cnt, :],
                                start=(j == 0),
                                stop=(j == 1),
                            )
                        pvz = zpool.tile([65, N], F32)
                        nc.scalar.copy(pvz[0:64, :], pv[0:64, :])
                        nc.vector.reciprocal(pvz[64:65, :], pv[64:65, :])
                        zb = ps_bc.tile([128, N], F32, tag="bc")
                        nc.tensor.matmul(
                            zb[0:64, :], ones[64:65, 0:64], pvz[64:65, :]
                        )
                        pvn = ppool.tile([128, N], F32, tag="pp")
                        nc.vector.tensor_tensor(
                            pvn[0:64, :], pvz[0:64, :], zb[0:64, :], op=OP.mult
                        )
                        nc.vector.copy_predicated(
                            at[hp][hb : hb + 64, bsl], tsb[b][0:64, :], pvn[0:64, :]
                        )

            # ---- proj + bias
            for ct in range(6):
                wps = []
                for f in range(6):
                    wt = w1pool.tile([128, 128], F32, tag="w")
                    nc.sync.dma_start(
                        wt[:],
                        wpt_d[f * 128 : (f + 1) * 128, ct * 128 : (ct + 1) * 128],
                    )
                    wps.append(wt)
                for tch in range(4):
                    sl = slice(tch * TCH, (tch + 1) * TCH)
                    pm = ps_mm.tile([128, TCH], F32, tag="mm")
                    for f in range(6):
                        nc.tensor.matmul(
                            pm[:], wps[f][:], at[f][:, sl],
                            start=(f == 0), stop=(f == 5),
                        )
                    ot = opool.tile([128, TCH], F32)
                    nc.scalar.activation(
                        ot[:], pm[:], AF.Identity, bias=bp_sb[ct][:]
                    )
                    nc.sync.dma_start(out_d[ct * 128 : (ct + 1) * 128, sl], ot[:])

    _split_multi_waits(nc)
    return nc


_NC = None


def _get_nc():
    global _NC
    if _NC is None:
        _NC = build_nc()
    return _NC


def make_in_maps(x, g1, g2, Wq, Wk, Wv, Wp, bp, Wm, bm):
    x = np.asarray(x, np.float32)
    g1 = np.asarray(g1, np.float32)
    g2 = np.asarray(g2, np.float32)
    wall = np.ascontiguousarray(
        np.concatenate(
            [np.asarray(Wq, np.float32), np.asarray(Wk, np.float32),
             np.asarray(Wv, np.float32), np.asarray(Wm, np.float32)], axis=0
        ).T
    )
    wpt = np.ascontiguousarray(np.asarray(Wp, np.float32).T)
    bpt = np.ascontiguousarray(np.asarray(bp, np.float32).reshape(6, 128))
    # gating threshold: sigmoid((l+g1-g2)/tau) > 0.6  <=>  l > tau*ln(1.5)-g1+g2
    # (bm == 0 in this problem's setup; fold it anyway)
    thr_tok = (
        5.0 * np.log(1.5) - g1[..., 0] + g2[..., 0] - np.asarray(bm, np.float32)[0]
    ).astype(np.float32)  # (B, N-1)
    in_maps = []
    for i in range(NCORES):
        xs = x[i * BL : (i + 1) * BL]  # (BL, N, C)
        xtp = np.ascontiguousarray(xs.reshape(T, C).T)
        thr = np.full((1, T), -1e30, np.float32)
        th = thr_tok[i * BL : (i + 1) * BL]  # (BL, N-1)
        thr_full = np.concatenate(
            [np.full((BL, 1), -1e30, np.float32), th], axis=1
        )  # (BL, N)
        thr[0] = thr_full.reshape(T)
        in_maps.append(
            {"xt": xtp, "wall": wall, "wpt": wpt, "bpt": bpt, "thr": thr}
        )
    return in_maps


def run(in_maps, trace=False, tmpdir=None):
    nc = _get_nc()
    return run_bass_kernel_spmd(
        nc, in_maps, core_ids=list(range(NCORES)), trace=trace, tmpdir=tmpdir
    )


def kernel(**inputs):
    res = run(make_in_maps(**inputs))
    outs = [
        res.results[i]["outt"].T.reshape(BL, N, C).astype(np.float32)
        for i in range(NCORES)
    ]
    return np.concatenate(outs, axis=0)


# revision 9
# speedup vs baseline: 1.6648x; 1.6648x over previous
"""AdaAttention (gumbel-gated sparse attention block) on 8 TRN2 NeuronCores.

Strategy: pure data-parallel over batch (64 batches -> 8 per core).  Each
core runs the full attention block for its 8 batches; no collectives.

Per-core layout (all f32):
  xt   [768, 1576]  x^T for this core's 8 batches (host transposes)
  wall [768, 2305]  concat([Wq, Wk, Wv, Wm], 0).T  (weights as lhsT tiles)
  wpt  [768, 768]   Wp.T
  bpt  [6, 128]     bp reshaped per 128-row output tile
  thr  [1, 1576]    gating threshold per token: 5*ln(1.5) - g1 + g2
                    (CLS slots = -1e30 so the CLS gate always passes)
  outt [768, 1576]  out^T (host transposes back)

Device pipeline:
  1. logits = Wm @ x^T (PE), ts = (logits > thr) via DVE is_gt; ts is
     round-tripped through DRAM to get per-partition column layout.
  2. QKV GEMM, transposed outputs: Q^T (pre-scaled by 1/8), K^T (spilled
     to DRAM, re-streamed per head-pair), V^T (doubles as the attention
     output buffer attnT: gated tokens pass v[n] through unchanged).
  3. V in normal layout ([token, head*65] with a ones column appended)
     for the PV matmul / softmax denominator.
  4. Per (batch, head): S^T = K_h @ Q_h^T (PE, odd/even heads land on
     row-groups 0/64 for concurrency); fused mask+exp in ONE ScalarE
     pass: exp(S*ts_m + 30*ts_m - 30) with per-partition scale/bias;
     PV^T+Z via [V|1] stationary; Z broadcast via a 1-row ones matmul;
     divide (DVE); copy_predicated overwrites kept-token columns of
     attnT (gated columns keep V^T).
  5. proj GEMM + bias, DMA out.
"""
import sys
import types

import numpy as np

# ---------------------------------------------------------------- patches
# This container's walrus rejects >1 sync-wait per instruction; Tile's
# kernel-tail drain aggregates one wait per outstanding proc.  Spread the
# waits across one sync-NOP each.  Also: the image's antenv lacks
# axon_hooks (NTFF profiling) and upload_artifacts wants a bucket.


def _install_patches():
    import bass_rust
    import concourse.tile as tile

    def _patched_drain_and_barrier(self, tick_clock, wait_clock):
        gc = tick_clock.global_clock
        ticks = eval(repr(gc).replace("VectorClock(", "").rstrip(")"))
        for i, t in enumerate(ticks):
            if t > 0:
                cur = list(ticks)
                cur[i] = 0
                nop = self.nc.sync.nop()
                wait_clock.add_sem_waits(
                    nop.ins,
                    tile.ScopedClock({None: gc}),
                    tile.ScopedClock({None: bass_rust.VectorClock(cur)}),
                )
        drain_inst = self.nc.sync.drain()
        wait_clock.add_sem_waits(
            drain_inst.ins, tile.ScopedClock({None: gc}), tile.ScopedClock({None: gc})
        )
        self.nc.all_engine_barrier()
        assert self.sems is not None
        popped = self.nc._tile_sem_poison_stack.pop()
        assert popped is self._sem_poison
        self.nc.clear_and_free_semaphores(list(self.sems.allocated().values()))
        self.nc.all_engine_barrier()

    tile.TileContext._drain_and_barrier = _patched_drain_and_barrier

    if "antenv.axon_hooks" not in sys.modules:
        mod = types.ModuleType("antenv.axon_hooks")
        try:
            from trn_agent_boot.trn_boot import _ntff_profile_via_ctypes

            hook = _ntff_profile_via_ctypes("/opt/axon/libaxon_pjrt.so")
        except Exception:
            hook = None
        mod.get_axon_ntff_profile_hook = lambda: hook
        mod.set_axon_ntff_profile_hook = lambda h: None
        sys.modules["antenv.axon_hooks"] = mod

    import concourse.bass_utils as bass_utils

    bass_utils.upload_artifacts = lambda tmpdir: f"file://{tmpdir}"


_install_patches()

import concourse.bass as bass  # noqa: E402
import concourse.mybir as mybir  # noqa: E402
import concourse.tile as tile  # noqa: E402
from concourse.bass_utils import run_bass_kernel_spmd  # noqa: E402


def _split_multi_waits(nc):
    """walrus here allows one sync-wait per engine instruction: hoist extra
    waits onto same-engine NoOps inserted immediately before."""
    for fn in nc.m.functions:
        for bb in fn.blocks:
            out = []
            changed = False
            for inst in bb.instructions:
                si = inst.sync_info
                waits = list(si.on_wait) if si is not None else []
                if len(waits) > 1:
                    changed = True
                    for k, w in enumerate(waits[:-1]):
                        nop = mybir.InstNoOp(
                            name=f"{inst.name}-w{k}",
                            engine=inst.engine,
                            ins=[],
                            outs=[],
                            sync_info=mybir.SyncInfo(on_wait=[w], on_update=[]),
                        )
                        out.append(nop)
                    si.on_wait = [waits[-1]]
                out.append(inst)
            if changed:
                try:
                    bb.instructions = out
                except Exception:
                    bb.instructions.clear()
                    bb.instructions.extend(out)

F32 = mybir.dt.float32
BF16 = mybir.dt.bfloat16
AF = mybir.ActivationFunctionType
OP = mybir.AluOpType

B, N, C = 64, 197, 768
H, D = 12, 64
NCORES = 8
BL = B // NCORES  # 8 batches per core
T = BL * N  # 1576 tokens per core
TCH = 394  # token chunk (= 2 batches); 4 chunks per core
NEG = 30.0  # masked scores get exp(-30)


def _chunks(b):
    """(start, count) partition chunks of batch b's 197 tokens."""
    a = b * N
    return [(a, 128), (a + 128, 69)]


def build_nc():
    nc = bass.Bass()
    xt_d = nc.declare_dram_parameter("xt", [C, T], F32, isOutput=False)
    wall_d = nc.declare_dram_parameter("wall", [C, 3 * C], BF16, isOutput=False)
    wmf_d = nc.declare_dram_parameter("wmf", [C, 1], F32, isOutput=False)
    wpt_d = nc.declare_dram_parameter("wpt", [C, C], BF16, isOutput=False)
    bpt_d = nc.declare_dram_parameter("bpt", [6, 128], F32, isOutput=False)
    thr_d = nc.declare_dram_parameter("thr", [1, T], F32, isOutput=False)
    out_d = nc.declare_dram_parameter("outt", [C, T], F32, isOutput=True)

    tsd = nc.dram_tensor("tsd", [1, T], F32, kind="Internal")

    with tile.TileContext(nc) as tc:
        from contextlib import ExitStack

        with ExitStack() as ctx:
            E = ctx.enter_context
            xpool = E(tc.tile_pool(name="xt", bufs=6))
            xbpool = E(tc.tile_pool(name="xb", bufs=6))
            qpool = E(tc.tile_pool(name="qt", bufs=6))
            kpool = E(tc.tile_pool(name="kt", bufs=6))
            apool = E(tc.tile_pool(name="at", bufs=6))
            rowpool = E(tc.tile_pool(name="rows", bufs=2))
            vpool = E(tc.tile_pool(name="vaug", bufs=16))
            w1pool = E(tc.tile_pool(name="w128", bufs=8))
            w3pool = E(tc.tile_pool(name="w384", bufs=7))
            ppool = E(tc.tile_pool(name="pp", bufs=8))
            rzpool = E(tc.tile_pool(name="rz", bufs=4))
            zbpool = E(tc.tile_pool(name="zbb", bufs=4))
            tbpool = E(tc.tile_pool(name="tsb", bufs=8))
            opool = E(tc.tile_pool(name="out", bufs=2))
            scpool = E(tc.tile_pool(name="smallcol", bufs=40))
            ps_mm = E(tc.tile_pool(name="ps_mm", bufs=2, space="PSUM"))
            ps_s = E(tc.tile_pool(name="ps_s", bufs=4, space="PSUM"))
            ps_pv = E(tc.tile_pool(name="ps_pv", bufs=2, space="PSUM"))

            # ---- constants / inputs
            ones = rzpool.tile([128, 128], F32, tag="ones")
            nc.vector.memset(ones[:], 1.0)

            xt = []
            for c in range(6):
                t_ = xpool.tile([128, T], F32)
                nc.sync.dma_start(t_[:], xt_d[c * 128 : (c + 1) * 128, :])
                xt.append(t_)

            thr_sb = rowpool.tile([1, T], F32, tag="rows")
            nc.sync.dma_start(thr_sb[:], thr_d[:])

            bp_sb = []
            for ct in range(6):
                t_ = scpool.tile([128, 1], F32, tag="bp")
                nc.sync.dma_start(
                    t_[:], bpt_d[ct : ct + 1, :].rearrange("a b -> b a")
                )
                bp_sb.append(t_)

            # ---- gating: logits = Wm @ x^T (f32!) ; ts = logits > thr
            wlog = []
            for c in range(6):
                t_ = scpool.tile([128, 1], F32, tag="wlog")
                nc.sync.dma_start(t_[:], wmf_d[c * 128 : (c + 1) * 128, :])
                wlog.append(t_)

            ts_row = rowpool.tile([1, T], F32, tag="rows")
            for tch in range(4):
                sl = slice(tch * TCH, (tch + 1) * TCH)
                pl = ps_s.tile([1, TCH], F32, tag="s")
                for c in range(6):
                    nc.tensor.matmul(
                        pl[:], wlog[c][:], xt[c][:, sl], start=(c == 0), stop=(c == 5)
                    )
                nc.vector.tensor_tensor(
                    ts_row[0:1, sl], pl[0:1, :], thr_sb[0:1, sl], op=OP.is_gt
                )
            nc.sync.dma_start(tsd[:], ts_row[:])

            ts_col, bias_col = [], []
            for b in range(BL):
                for (a, cnt) in _chunks(b):
                    tcl = scpool.tile([128, 1], F32, tag="tscol")
                    nc.sync.dma_start(
                        tcl[0:cnt, :], tsd[0:1, a : a + cnt].rearrange("a b -> b a")
                    )
                    bcl = scpool.tile([128, 1], F32, tag="bcol")
                    nc.vector.tensor_scalar(
                        bcl[0:cnt, :], tcl[0:cnt, :], NEG, -NEG, op0=OP.mult, op1=OP.add
                    )
                    ts_col.append(tcl)
                    bias_col.append(bcl)

            # ts broadcast [128,197] per batch -> int32 predicate for the blend
            tsb = []
            for b in range(BL):
                pt = ps_pv.tile([128, N], F32, tag="pv")
                nc.tensor.matmul(
                    pt[:], ones[0:1, 0:128], ts_row[0:1, b * N : (b + 1) * N]
                )
                st = tbpool.tile([128, N], mybir.dt.int32)
                nc.vector.tensor_copy(st[:], pt[:])
                tsb.append(st)

            # ---- x in bf16 for all TensorE work
            xb = []
            for c in range(6):
                t_ = xbpool.tile([128, T], BF16)
                if c % 2 == 0:
                    nc.vector.tensor_copy(t_[:], xt[c][:])
                else:
                    nc.scalar.copy(t_[:], xt[c][:])
                xb.append(t_)

            # ---- QKV GEMM (transposed outputs).  f-tiles: 0-5 Q (pre-scaled
            # by 1/8), 6-11 K, 12-17 V^T (V^T tiles double as attnT).
            qt, kt, at = [], [], []
            for f in range(18):
                wts = []
                for c in range(6):
                    wt = w1pool.tile([128, 128], BF16, tag="w")
                    nc.sync.dma_start(
                        wt[:],
                        wall_d[c * 128 : (c + 1) * 128, f * 128 : (f + 1) * 128],
                    )
                    wts.append(wt)
                if f < 6:
                    dst = qpool.tile([128, T], BF16)
                    qt.append(dst)
                elif f < 12:
                    dst = kpool.tile([128, T], BF16)
                    kt.append(dst)
                else:
                    dst = apool.tile([128, T], BF16)
                    at.append(dst)
                for tch in range(4):
                    sl = slice(tch * TCH, (tch + 1) * TCH)
                    pm = ps_mm.tile([128, TCH], F32, tag="mm")
                    for c in range(6):
                        nc.tensor.matmul(
                            pm[:], wts[c][:], xb[c][:, sl],
                            start=(c == 0), stop=(c == 5),
                        )
                    if f < 6:
                        nc.vector.tensor_scalar_mul(dst[:, sl], pm[:], 0.125)
                    elif (f + tch) % 2 == 0:
                        nc.scalar.copy(dst[:, sl], pm[:])
                    else:
                        nc.vector.tensor_copy(dst[:, sl], pm[:])

            # ---- V in normal layout, ones column LAST (Z lands on
            # partition 64 of the PV matmul output; 64 is 32-aligned):
            # vaug[b,j][m, h, 0:64] = V, [m, h, 64] = 1.0
            vaug = []
            for b in range(BL):
                for j, (a, cnt) in enumerate(_chunks(b)):
                    vt = vpool.tile([128, H, D + 1], BF16)
                    nc.vector.memset(vt[:, :, D : D + 1], 1.0)
                    vaug.append(vt)
            for n2 in range(2):
                wv = []
                for c in range(6):
                    wt = w3pool.tile([128, 384], BF16, tag="wv")
                    nc.sync.dma_start(
                        wt[:],
                        wall_d[
                            c * 128 : (c + 1) * 128,
                            2 * C + n2 * 384 : 2 * C + (n2 + 1) * 384,
                        ],
                    )
                    wv.append(wt)
                for b in range(BL):
                    for j, (a, cnt) in enumerate(_chunks(b)):
                        pm = ps_mm.tile([128, TCH], F32, tag="mm")
                        for c in range(6):
                            nc.tensor.matmul(
                                pm[0:cnt, 0:384],
                                xb[c][:, a : a + cnt],
                                wv[c][:],
                                start=(c == 0),
                                stop=(c == 5),
                            )
                        nc.scalar.copy(
                            vaug[b * 2 + j][0:cnt, n2 * 6 : (n2 + 1) * 6, 0:D],
                            pm[0:cnt, 0:384].rearrange("p (h d) -> p h d", d=D),
                        )

            # ---- attention, batch-major (everything SBUF-resident)
            for b in range(BL):
                bsl = slice(b * N, (b + 1) * N)
                for h in range(H):
                    hp, hb = h // 2, (h % 2) * 64
                    pps = []
                    for j, (a, cnt) in enumerate(_chunks(b)):
                        ps = ps_s.tile([128, N], F32, tag="s")
                        nc.tensor.matmul(
                            ps[0:cnt, :],
                            kt[hp][hb : hb + 64, a : a + cnt],
                            qt[hp][hb : hb + 64, bsl],
                        )
                        pp = ppool.tile([128, N], BF16, tag="pp")
                        nc.scalar.activation(
                            pp[0:cnt, :],
                            ps[0:cnt, :],
                            AF.Exp,
                            bias=bias_col[b * 2 + j][0:cnt, :],
                            scale=ts_col[b * 2 + j][0:cnt, :],
                        )
                        pps.append(pp)
                    pv = ps_pv.tile([65, N], F32, tag="pv")
                    for j, (a, cnt) in enumerate(_chunks(b)):
                        nc.tensor.matmul(
                            pv[:],
                            vaug[b * 2 + j][0:cnt, h, :],
                            pps[j][0:cnt, :],
                            start=(j == 0),
                            stop=(j == 1),
                        )
                    rz = rzpool.tile([1, N], F32, tag="rz")
                    nc.vector.reciprocal(rz[0:1, :], pv[64:65, :])
                    pvsb = zbpool.tile([65, N], F32, tag="zbb")
                    nc.scalar.copy(pvsb[0:64, :], pv[0:64, :])
                    zb = ps_s.tile([128, N], F32, tag="s")
                    nc.tensor.matmul(zb[0:64, :], ones[0:1, 0:64], rz[0:1, :])
                    pvn = ppool.tile([128, N], BF16, tag="pp")
                    nc.vector.tensor_tensor(
                        pvn[0:64, :], pvsb[0:64, :], zb[0:64, :], op=OP.mult
                    )
                    nc.vector.copy_predicated(
                        at[hp][hb : hb + 64, bsl], tsb[b][0:64, :], pvn[0:64, :]
                    )

            # ---- proj + bias
            for ct in range(6):
                wps = []
                for f in range(6):
                    wt = w1pool.tile([128, 128], BF16, tag="w")
                    nc.sync.dma_start(
                        wt[:],
                        wpt_d[f * 128 : (f + 1) * 128, ct * 128 : (ct + 1) * 128],
                    )
                    wps.append(wt)
                for tch in range(4):
                    sl = slice(tch * TCH, (tch + 1) * TCH)
                    pm = ps_mm.tile([128, TCH], F32, tag="mm")
                    for f in range(6):
                        nc.tensor.matmul(
                            pm[:], wps[f][:], at[f][:, sl],
                            start=(f == 0), stop=(f == 5),
                        )
                    ot = opool.tile([128, TCH], F32)
                    nc.scalar.activation(
                        ot[:], pm[:], AF.Identity, bias=bp_sb[ct][:]
                    )
                    nc.sync.dma_start(out_d[ct * 128 : (ct + 1) * 128, sl], ot[:])

    _split_multi_waits(nc)
    return nc


_NC = None


def _get_nc():
    global _NC
    if _NC is None:
        _NC = build_nc()
    return _NC


def make_in_maps(x, g1, g2, Wq, Wk, Wv, Wp, bp, Wm, bm):
    import ml_dtypes

    bf16 = ml_dtypes.bfloat16
    x = np.asarray(x, np.float32)
    g1 = np.asarray(g1, np.float32)
    g2 = np.asarray(g2, np.float32)
    wall = np.ascontiguousarray(
        np.concatenate(
            [np.asarray(Wq, np.float32), np.asarray(Wk, np.float32),
             np.asarray(Wv, np.float32)], axis=0
        ).T.astype(bf16)
    )
    wmf = np.ascontiguousarray(np.asarray(Wm, np.float32).T)  # (C, 1)
    wpt = np.ascontiguousarray(np.asarray(Wp, np.float32).T.astype(bf16))
    bpt = np.ascontiguousarray(np.asarray(bp, np.float32).reshape(6, 128))
    # gating threshold: sigmoid((l+g1-g2)/tau) > 0.6  <=>  l > tau*ln(1.5)-g1+g2
    # (bm == 0 in this problem's setup; fold it anyway)
    thr_tok = (
        5.0 * np.log(1.5) - g1[..., 0] + g2[..., 0] - np.asarray(bm, np.float32)[0]
    ).astype(np.float32)  # (B, N-1)
    in_maps = []
    for i in range(NCORES):
        xs = x[i * BL : (i + 1) * BL]  # (BL, N, C)
        xtp = np.ascontiguousarray(xs.reshape(T, C).T)
        thr = np.full((1, T), -1e30, np.float32)
        th = thr_tok[i * BL : (i + 1) * BL]  # (BL, N-1)
        thr_full = np.concatenate(
            [np.full((BL, 1), -1e30, np.float32), th], axis=1
        )  # (BL, N)
        thr[0] = thr_full.reshape(T)
        in_maps.append(
            {"xt": xtp, "wall": wall, "wmf": wmf, "wpt": wpt, "bpt": bpt,
             "thr": thr}
        )
    return in_maps


def run(in_maps, trace=False, tmpdir=None):
    nc = _get_nc()
    return run_bass_kernel_spmd(
        nc, in_maps, core_ids=list(range(NCORES)), trace=trace, tmpdir=tmpdir
    )


def kernel(**inputs):
    res = run(make_in_maps(**inputs))
    outs = [
        res.results[i]["outt"].T.reshape(BL, N, C).astype(np.float32)
        for i in range(NCORES)
    ]
    return np.concatenate(outs, axis=0)


# revision 15
# speedup vs baseline: 2.0865x; 1.2533x over previous
"""AdaAttention (gumbel-gated sparse attention block) on 8 TRN2 NeuronCores.

Strategy: pure data-parallel over batch (64 batches -> 8 per core).  Each
core runs the full attention block for its 8 batches; no collectives.

Per-core layout (all f32):
  xt   [768, 1576]  x^T for this core's 8 batches (host transposes)
  wall [768, 2305]  concat([Wq, Wk, Wv, Wm], 0).T  (weights as lhsT tiles)
  wpt  [768, 768]   Wp.T
  bpt  [6, 128]     bp reshaped per 128-row output tile
  thr  [1, 1576]    gating threshold per token: 5*ln(1.5) - g1 + g2
                    (CLS slots = -1e30 so the CLS gate always passes)
  outt [768, 1576]  out^T (host transposes back)

Device pipeline:
  1. logits = Wm @ x^T (PE), ts = (logits > thr) via DVE is_gt; ts is
     round-tripped through DRAM to get per-partition column layout.
  2. QKV GEMM, transposed outputs: Q^T (pre-scaled by 1/8), K^T (spilled
     to DRAM, re-streamed per head-pair), V^T (doubles as the attention
     output buffer attnT: gated tokens pass v[n] through unchanged).
  3. V in normal layout ([token, head*65] with a ones column appended)
     for the PV matmul / softmax denominator.
  4. Per (batch, head): S^T = K_h @ Q_h^T (PE, odd/even heads land on
     row-groups 0/64 for concurrency); fused mask+exp in ONE ScalarE
     pass: exp(S*ts_m + 30*ts_m - 30) with per-partition scale/bias;
     PV^T+Z via [V|1] stationary; Z broadcast via a 1-row ones matmul;
     divide (DVE); copy_predicated overwrites kept-token columns of
     attnT (gated columns keep V^T).
  5. proj GEMM + bias, DMA out.
"""
import sys
import types

import numpy as np

# ---------------------------------------------------------------- patches
# This container's walrus rejects >1 sync-wait per instruction; Tile's
# kernel-tail drain aggregates one wait per outstanding proc.  Spread the
# waits across one sync-NOP each.  Also: the image's antenv lacks
# axon_hooks (NTFF profiling) and upload_artifacts wants a bucket.


def _install_patches():
    import bass_rust
    import concourse.tile as tile

    def _patched_drain_and_barrier(self, tick_clock, wait_clock):
        gc = tick_clock.global_clock
        ticks = eval(repr(gc).replace("VectorClock(", "").rstrip(")"))
        for i, t in enumerate(ticks):
            if t > 0:
                cur = list(ticks)
                cur[i] = 0
                nop = self.nc.sync.nop()
                wait_clock.add_sem_waits(
                    nop.ins,
                    tile.ScopedClock({None: gc}),
                    tile.ScopedClock({None: bass_rust.VectorClock(cur)}),
                )
        drain_inst = self.nc.sync.drain()
        wait_clock.add_sem_waits(
            drain_inst.ins, tile.ScopedClock({None: gc}), tile.ScopedClock({None: gc})
        )
        self.nc.all_engine_barrier()
        assert self.sems is not None
        popped = self.nc._tile_sem_poison_stack.pop()
        assert popped is self._sem_poison
        self.nc.clear_and_free_semaphores(list(self.sems.allocated().values()))
        self.nc.all_engine_barrier()

    tile.TileContext._drain_and_barrier = _patched_drain_and_barrier

    if "antenv.axon_hooks" not in sys.modules:
        mod = types.ModuleType("antenv.axon_hooks")
        try:
            from trn_agent_boot.trn_boot import _ntff_profile_via_ctypes

            hook = _ntff_profile_via_ctypes("/opt/axon/libaxon_pjrt.so")
        except Exception:
            hook = None
        mod.get_axon_ntff_profile_hook = lambda: hook
        mod.set_axon_ntff_profile_hook = lambda h: None
        sys.modules["antenv.axon_hooks"] = mod

    import concourse.bass_utils as bass_utils

    bass_utils.upload_artifacts = lambda tmpdir: f"file://{tmpdir}"


_install_patches()

import concourse.bass as bass  # noqa: E402
import concourse.mybir as mybir  # noqa: E402
import concourse.tile as tile  # noqa: E402
from concourse.bass_utils import run_bass_kernel_spmd  # noqa: E402


def _split_multi_waits(nc):
    """walrus here allows one sync-wait per engine instruction: hoist extra
    waits onto same-engine NoOps inserted immediately before."""
    for fn in nc.m.functions:
        for bb in fn.blocks:
            out = []
            changed = False
            for inst in bb.instructions:
                si = inst.sync_info
                waits = list(si.on_wait) if si is not None else []
                if len(waits) > 1:
                    changed = True
                    for k, w in enumerate(waits[:-1]):
                        nop = mybir.InstNoOp(
                            name=f"{inst.name}-w{k}",
                            engine=inst.engine,
                            ins=[],
                            outs=[],
                            sync_info=mybir.SyncInfo(on_wait=[w], on_update=[]),
                        )
                        out.append(nop)
                    si.on_wait = [waits[-1]]
                out.append(inst)
            if changed:
                try:
                    bb.instructions = out
                except Exception:
                    bb.instructions.clear()
                    bb.instructions.extend(out)

F32 = mybir.dt.float32
BF16 = mybir.dt.bfloat16
AF = mybir.ActivationFunctionType
OP = mybir.AluOpType

B, N, C = 64, 197, 768
H, D = 12, 64
NCORES = 8
BL = B // NCORES  # 8 batches per core
T = BL * N  # 1576 tokens per core
TCH = 394  # token chunk (= 2 batches); 4 chunks per core
NEG = 30.0  # masked scores get exp(-30)


def _chunks(b):
    """(start, count) partition chunks of batch b's 197 tokens."""
    a = b * N
    return [(a, 128), (a + 128, 69)]


def build_nc():
    nc = bass.Bass()
    xt_d = nc.declare_dram_parameter("xt", [C, T], F32, isOutput=False)
    wall_d = nc.declare_dram_parameter("wall", [C, 3 * C], BF16, isOutput=False)
    wmf_d = nc.declare_dram_parameter("wmf", [C, 1], F32, isOutput=False)
    wpt_d = nc.declare_dram_parameter("wpt", [C, C], BF16, isOutput=False)
    bpt_d = nc.declare_dram_parameter("bpt", [6, 128], F32, isOutput=False)
    thr_d = nc.declare_dram_parameter("thr", [1, T], F32, isOutput=False)
    out_d = nc.declare_dram_parameter("outt", [C, T], F32, isOutput=True)

    tsd = nc.dram_tensor("tsd", [1, T], F32, kind="Internal")
    rzd = nc.dram_tensor("rzd", [24, 4, N], F32, kind="Internal")

    with tile.TileContext(nc) as tc:
        from contextlib import ExitStack

        with ExitStack() as ctx:
            E = ctx.enter_context
            xpool = E(tc.tile_pool(name="xt", bufs=6))
            xbpool = E(tc.tile_pool(name="xb", bufs=6))
            qpool = E(tc.tile_pool(name="qt", bufs=6))
            kpool = E(tc.tile_pool(name="kt", bufs=6))
            apool = E(tc.tile_pool(name="at", bufs=6))
            rowpool = E(tc.tile_pool(name="rows", bufs=2))
            vpool = E(tc.tile_pool(name="vn", bufs=16))
            w1pool = E(tc.tile_pool(name="w128", bufs=8))
            w3pool = E(tc.tile_pool(name="w384", bufs=7))
            wppool = E(tc.tile_pool(name="wp", bufs=36))
            ppool = E(tc.tile_pool(name="pp", bufs=10))
            rzpool = E(tc.tile_pool(name="rz", bufs=2))
            zbpool = E(tc.tile_pool(name="zbb", bufs=4))
            tbpool = E(tc.tile_pool(name="tsb", bufs=8))
            opool = E(tc.tile_pool(name="out", bufs=3))
            scpool = E(tc.tile_pool(name="smallcol", bufs=40))
            ps_mm = E(tc.tile_pool(name="ps_mm", bufs=2, space="PSUM"))
            ps_s = E(tc.tile_pool(name="ps_s", bufs=3, space="PSUM"))
            ps_pv = E(tc.tile_pool(name="ps_pv", bufs=2, space="PSUM"))
            ps_z = E(tc.tile_pool(name="ps_z", bufs=1, space="PSUM"))

            # ---- constants / inputs
            ones_bf = scpool.tile([128, 1], BF16, tag="onesb")
            nc.vector.memset(ones_bf[:], 1.0)

            xt = []
            for c in range(6):
                t_ = xpool.tile([128, T], F32)
                nc.sync.dma_start(t_[:], xt_d[c * 128 : (c + 1) * 128, :])
                xt.append(t_)

            thr_sb = rowpool.tile([1, T], F32, tag="rows")
            nc.sync.dma_start(thr_sb[:], thr_d[:])

            bp_sb = []
            for ct in range(6):
                t_ = scpool.tile([128, 1], F32, tag="bp")
                nc.sync.dma_start(
                    t_[:], bpt_d[ct : ct + 1, :].rearrange("a b -> b a")
                )
                bp_sb.append(t_)

            # ---- gating: logits = Wm @ x^T (f32!) ; ts = logits > thr
            wlog = []
            for c in range(6):
                t_ = scpool.tile([128, 1], F32, tag="wlog")
                nc.sync.dma_start(t_[:], wmf_d[c * 128 : (c + 1) * 128, :])
                wlog.append(t_)

            ts_row = rowpool.tile([1, T], F32, tag="rows")
            for tch in range(4):
                sl = slice(tch * TCH, (tch + 1) * TCH)
                pl = ps_s.tile([1, TCH], F32, tag="s")
                for c in range(6):
                    nc.tensor.matmul(
                        pl[:], wlog[c][:], xt[c][:, sl], start=(c == 0), stop=(c == 5)
                    )
                nc.vector.tensor_tensor(
                    ts_row[0:1, sl], pl[0:1, :], thr_sb[0:1, sl], op=OP.is_gt
                )
            nc.sync.dma_start(tsd[:], ts_row[:])

            ts_col, bias_col = [], []
            for b in range(BL):
                for (a, cnt) in _chunks(b):
                    tcl = scpool.tile([128, 1], F32, tag="tscol")
                    nc.sync.dma_start(
                        tcl[0:cnt, :], tsd[0:1, a : a + cnt].rearrange("a b -> b a")
                    )
                    bcl = scpool.tile([128, 1], F32, tag="bcol")
                    nc.vector.tensor_scalar(
                        bcl[0:cnt, :], tcl[0:cnt, :], NEG, -NEG, op0=OP.mult, op1=OP.add
                    )
                    ts_col.append(tcl)
                    bias_col.append(bcl)

            # ts broadcast [128,197] per batch via DMA (step-0 partition AP),
            # then cast to the int32 predicate the blend needs
            tsb = []
            for b in range(BL):
                tf = zbpool.tile([128, N], F32, tag="zbb")
                nc.sync.dma_start(
                    tf[:], tsd[0:1, b * N : (b + 1) * N].to_broadcast((128, N))
                )
                st = tbpool.tile([128, N], mybir.dt.int32)
                nc.vector.tensor_copy(st[:], tf[:])
                tsb.append(st)

            # ---- x in bf16 for all TensorE work
            xb = []
            for c in range(6):
                t_ = xbpool.tile([128, T], BF16)
                if c % 2 == 0:
                    nc.vector.tensor_copy(t_[:], xt[c][:])
                else:
                    nc.scalar.copy(t_[:], xt[c][:])
                xb.append(t_)

            # ---- QKV GEMM (transposed outputs).  f-tiles: 0-5 Q (pre-scaled
            # by 1/8), 6-11 K, 12-17 V^T (V^T tiles double as attnT).
            qt, kt, at = [], [], []
            for f in range(18):
                wts = []
                for c in range(6):
                    wt = w1pool.tile([128, 128], BF16, tag="w")
                    nc.sync.dma_start(
                        wt[:],
                        wall_d[c * 128 : (c + 1) * 128, f * 128 : (f + 1) * 128],
                    )
                    wts.append(wt)
                if f < 6:
                    dst = qpool.tile([128, T], BF16)
                    qt.append(dst)
                elif f < 12:
                    dst = kpool.tile([128, T], BF16)
                    kt.append(dst)
                else:
                    dst = apool.tile([128, T], BF16)
                    at.append(dst)
                for tch in range(4):
                    sl = slice(tch * TCH, (tch + 1) * TCH)
                    pm = ps_mm.tile([128, TCH], F32, tag="mm")
                    for c in range(6):
                        nc.tensor.matmul(
                            pm[:], wts[c][:], xb[c][:, sl],
                            start=(c == 0), stop=(c == 5),
                        )
                    if f < 6:
                        nc.vector.tensor_scalar_mul(dst[:, sl], pm[:], 0.125)
                    elif (f + tch) % 2 == 0:
                        nc.scalar.copy(dst[:, sl], pm[:])
                    else:
                        nc.vector.tensor_copy(dst[:, sl], pm[:])

            # ---- V in normal (token-partition) layout
            vn = []
            for b in range(BL):
                for j, (a, cnt) in enumerate(_chunks(b)):
                    vn.append(vpool.tile([128, C], BF16, name="vn", tag="vn"))
            for n2 in range(2):
                wv = []
                for c in range(6):
                    wt = w3pool.tile([128, 384], BF16, tag="wv")
                    nc.sync.dma_start(
                        wt[:],
                        wall_d[
                            c * 128 : (c + 1) * 128,
                            2 * C + n2 * 384 : 2 * C + (n2 + 1) * 384,
                        ],
                    )
                    wv.append(wt)
                for b in range(BL):
                    for j, (a, cnt) in enumerate(_chunks(b)):
                        pm = ps_mm.tile([128, TCH], F32, tag="mm")
                        for c in range(6):
                            nc.tensor.matmul(
                                pm[0:cnt, 0:384],
                                xb[c][:, a : a + cnt],
                                wv[c][:],
                                start=(c == 0),
                                stop=(c == 5),
                            )
                        nc.scalar.copy(
                            vn[b * 2 + j][0:cnt, n2 * 384 : (n2 + 1) * 384],
                            pm[0:cnt, 0:384],
                        )

            # proj weights, persistent
            wps = {}
            for ct in range(6):
                for f in range(6):
                    wt = wppool.tile([128, 128], BF16, tag="wp")
                    nc.sync.dma_start(
                        wt[:],
                        wpt_d[f * 128 : (f + 1) * 128, ct * 128 : (ct + 1) * 128],
                    )
                    wps[(f, ct)] = wt

            # ---- attention, batch-major; per batch: 3 quads of 4 heads.
            # Head pairs (2hp, 2hp+1) pack into one [128,197] PV psum tile;
            # Z sums land on 32-aligned rows of one psum tile per quad ->
            # a single DVE reciprocal per quad.  1/Z rows are broadcast to
            # [64,197] via SBUF->SBUF DMA with a step-0 partition AP.
            for b in range(BL):
                bsl = slice(b * N, (b + 1) * N)
                for q in range(3):
                    pps = {}
                    zps = ps_z.tile([97, N], F32, tag="z")
                    pvs = []
                    for pair in range(2):
                        for hh in range(2):
                            k = 2 * pair + hh
                            h = 4 * q + k
                            hp, hb = h // 2, (h % 2) * 64
                            for j, (a, cnt) in enumerate(_chunks(b)):
                                ps = ps_s.tile([128, N], F32, tag="s")
                                nc.tensor.matmul(
                                    ps[0:cnt, :],
                                    kt[hp][hb : hb + 64, a : a + cnt],
                                    qt[hp][hb : hb + 64, bsl],
                                )
                                pp = ppool.tile([128, N], BF16, tag="pp")
                                nc.scalar.activation(
                                    pp[0:cnt, :],
                                    ps[0:cnt, :],
                                    AF.Exp,
                                    bias=bias_col[b * 2 + j][0:cnt, :],
                                    scale=ts_col[b * 2 + j][0:cnt, :],
                                )
                                pps[(k, j)] = pp
                    for pair in range(2):
                        pv2 = ps_pv.tile([128, N], F32, tag="pv")
                        pvs.append(pv2)
                        for hh in range(2):
                            k = 2 * pair + hh
                            h = 4 * q + k
                            for j, (a, cnt) in enumerate(_chunks(b)):
                                nc.tensor.matmul(
                                    pv2[64 * hh : 64 * hh + 64, :],
                                    vn[b * 2 + j][0:cnt, h * 64 : (h + 1) * 64],
                                    pps[(k, j)][0:cnt, :],
                                    start=(j == 0),
                                    stop=(j == 1),
                                )
                            for j, (a, cnt) in enumerate(_chunks(b)):
                                nc.tensor.matmul(
                                    zps[32 * k : 32 * k + 1, :],
                                    ones_bf[0:cnt, :],
                                    pps[(k, j)][0:cnt, :],
                                    start=(j == 0),
                                    stop=(j == 1),
                                    tile_position=(0, 32 * k),
                                )
                    rzq = rzpool.tile([97, N], F32, tag="rz")
                    nc.vector.reciprocal(rzq[:], zps[:])
                    qq = b * 3 + q
                    nc.sync.dma_start(
                        rzd[qq, :, :], rzq[0 : 97 : 32, :]
                    )
                    for pair in range(2):
                        h0 = 4 * q + 2 * pair
                        hp = h0 // 2
                        zbb = zbpool.tile([128, N], F32, tag="zbb")
                        for hh in range(2):
                            k = 2 * pair + hh
                            nc.sync.dma_start(
                                zbb[64 * hh : 64 * hh + 64, :],
                                rzd[qq, k : k + 1, :].to_broadcast((64, N)),
                            )
                        pvn2 = ppool.tile([128, N], BF16, tag="pp")
                        nc.vector.tensor_tensor(
                            pvn2[:], pvs[pair][:], zbb[:], op=OP.mult
                        )
                        nc.vector.copy_predicated(
                            at[hp][:, bsl], tsb[b][:], pvn2[:]
                        )

                # ---- proj for this pair of batches, interleaved to keep
                # the PE dense (t-chunk = 2 batches)
                if b % 2 == 1:
                    tch = b // 2
                    sl = slice(tch * TCH, (tch + 1) * TCH)
                    for ct in range(6):
                        pm = ps_mm.tile([128, TCH], F32, tag="mm")
                        for f in range(6):
                            nc.tensor.matmul(
                                pm[:], wps[(f, ct)][:], at[f][:, sl],
                                start=(f == 0), stop=(f == 5),
                            )
                        ot = opool.tile([128, TCH], F32)
                        nc.scalar.activation(
                            ot[:], pm[:], AF.Identity, bias=bp_sb[ct][:]
                        )
                        nc.sync.dma_start(
                            out_d[ct * 128 : (ct + 1) * 128, sl], ot[:]
                        )

    _split_multi_waits(nc)
    return nc


_NC = None


def _get_nc():
    global _NC
    if _NC is None:
        _NC = build_nc()
    return _NC


def make_in_maps(x, g1, g2, Wq, Wk, Wv, Wp, bp, Wm, bm):
    import ml_dtypes

    bf16 = ml_dtypes.bfloat16
    x = np.asarray(x, np.float32)
    g1 = np.asarray(g1, np.float32)
    g2 = np.asarray(g2, np.float32)
    wall = np.ascontiguousarray(
        np.concatenate(
            [np.asarray(Wq, np.float32), np.asarray(Wk, np.float32),
             np.asarray(Wv, np.float32)], axis=0
        ).T.astype(bf16)
    )
    wmf = np.ascontiguousarray(np.asarray(Wm, np.float32).T)  # (C, 1)
    wpt = np.ascontiguousarray(np.asarray(Wp, np.float32).T.astype(bf16))
    bpt = np.ascontiguousarray(np.asarray(bp, np.float32).reshape(6, 128))
    # gating threshold: sigmoid((l+g1-g2)/tau) > 0.6  <=>  l > tau*ln(1.5)-g1+g2
    # (bm == 0 in this problem's setup; fold it anyway)
    thr_tok = (
        5.0 * np.log(1.5) - g1[..., 0] + g2[..., 0] - np.asarray(bm, np.float32)[0]
    ).astype(np.float32)  # (B, N-1)
    in_maps = []
    for i in range(NCORES):
        xs = x[i * BL : (i + 1) * BL]  # (BL, N, C)
        xtp = np.ascontiguousarray(xs.reshape(T, C).T)
        thr = np.full((1, T), -1e30, np.float32)
        th = thr_tok[i * BL : (i + 1) * BL]  # (BL, N-1)
        thr_full = np.concatenate(
            [np.full((BL, 1), -1e30, np.float32), th], axis=1
        )  # (BL, N)
        thr[0] = thr_full.reshape(T)
        in_maps.append(
            {"xt": xtp, "wall": wall, "wmf": wmf, "wpt": wpt, "bpt": bpt,
             "thr": thr}
        )
    return in_maps


def run(in_maps, trace=False, tmpdir=None):
    nc = _get_nc()
    return run_bass_kernel_spmd(
        nc, in_maps, core_ids=list(range(NCORES)), trace=trace, tmpdir=tmpdir
    )


def kernel(**inputs):
    res = run(make_in_maps(**inputs))
    outs = [
        res.results[i]["outt"].T.reshape(BL, N, C).astype(np.float32)
        for i in range(NCORES)
    ]
    return np.concatenate(outs, axis=0)
